# revision 9
# baseline (speedup 1.0000x reference)
"""CURLoRA layer kernel for 8 TRN2 NeuronCores.

Computes out = x @ (W + C@U@R)^T + bias for
  x: (4, 2048, 4096) f32, W: (4096, 4096), C: (4096, 64), U: (64, 64),
  R: (64, 4096), bias: (4096,)  ->  out: (4, 2048, 4096) f32

Sharding: 8 cores = 2 token-groups x 4 output-column-groups.
Each core computes out[tg, og] = x[tg] @ (W[og] + C[og]@U@R)^T + bias[og]
independently (no collectives needed).

Per-core kernel (bf16 compute, fp32 accumulate). Inputs are pre-cast to
bf16 on the host, which halves HBM reads and lets the DMA xbar engine do
all transposes, keeping the PE array on a pure matmul stream:
  1. W'^T built in SBUF: W^T k-slices arrive via dma_start_transpose,
     adapter (C@U@R)^T = R^T-stationary @ (U^T C^T) via PE (K=64), DVE
     adds them into the resident wt_sb [128d, 32k, 1024o] bf16.
  2. x^T tiles arrive via dma_start_transpose in 512-token chunks,
     double-buffered: xt [128d, 32k, 512t] bf16.
  3. Main stream: per t-tile, 2 x 32 matmuls accumulate into PSUM;
     DVE adds bias on eviction; SWDGE writes out.
"""

import sys

if "/opt/trn_rl_repo" not in sys.path:
    sys.path.insert(0, "/opt/trn_rl_repo")

import numpy as np
import ml_dtypes

B, S, D = 4, 2048, 4096
O = 4096
RK = 64
T = B * S  # 8192 tokens
NT, NO = 2, 4  # token groups x out-column groups
TSH = T // NT  # 4096 tokens per core
OSH = O // NO  # 1024 out columns per core
N_CORES = 8

NK = D // 128  # 32 k-tiles
TC = 512  # x^T chunk tokens
NCH = TSH // TC  # 8 chunks
NTT_C = TC // 128  # 4 t-tiles per chunk
NJ = OSH // 512  # 2 o-blocks of 512

_CACHE = {}


def _build():
    from concourse import bacc
    import concourse.bass as bass
    import concourse.mybir as mybir
    from concourse.bass import ts
    from concourse.tile import TileContext
    from concourse.masks import make_identity

    f32 = mybir.dt.float32
    bf16 = mybir.dt.bfloat16

    nc = bacc.Bacc()
    x_ext = nc.declare_dram_parameter("x", [TSH, D], bf16, isOutput=False)
    w_ext = nc.declare_dram_parameter("W", [OSH, D], bf16, isOutput=False)
    c_ext = nc.declare_dram_parameter("C", [OSH, 128], bf16, isOutput=False)
    u_ext = nc.declare_dram_parameter("U", [RK, RK], bf16, isOutput=False)
    r_ext = nc.declare_dram_parameter("R", [RK, D], bf16, isOutput=False)
    b_ext = nc.declare_dram_parameter("bias", [OSH], f32, isOutput=False)
    out_ext = nc.declare_dram_parameter("out", [TSH, OSH], f32, isOutput=True)
    import os as _os
    _DBG = _os.environ.get("KDBG", "") == "1"
    if _DBG:
        dbg_wt = nc.declare_dram_parameter("dbg_wt", [128, NK * OSH], f32, isOutput=True)
        dbg_xt = nc.declare_dram_parameter("dbg_xt", [128, NK * TC], f32, isOutput=True)

    with TileContext(nc) as tc:
        with (
            tc.tile_pool(name="const", bufs=1) as const,
            tc.tile_pool(name="wt", bufs=1) as wtp,
            tc.tile_pool(name="small", bufs=1) as small,
            tc.tile_pool(name="wstage", bufs=2) as wsp,
            tc.tile_pool(name="xtpool", bufs=1) as xtpool,
            tc.tile_pool(name="opool", bufs=2) as opool,
            # PSUM: psA (ad 3 + warm 1) + psB (out 4) = 8 banks
            tc.tile_pool(name="psA", bufs=4, space="PSUM") as psA,
            tc.tile_pool(name="psB", bufs=4, space="PSUM") as psB,
        ):
            ident = const.tile([128, 128], bf16)
            make_identity(nc, ident)
            cst = const.tile([128, 512], bf16)
            for q in range(4):
                nc.vector.tensor_copy(out=cst[:, ts(q, 128)], in_=ident[:])

            # resident W'^T: [128 d-part, 32 k-tiles, 1024 o] bf16
            wt_sb = wtp.tile([128, NK, OSH], bf16)
            bias_sb = const.tile([128, OSH], f32)

            # small inputs on the SWDGE queue (gpsimd); transposes on HWDGE
            u_sb = small.tile([RK, RK], bf16)
            nc.gpsimd.dma_start(out=u_sb[:], in_=u_ext[:])
            r_sb = small.tile([RK, D], bf16)
            nc.gpsimd.dma_start(out=r_sb[:], in_=r_ext[:])
            ct_sb = small.tile([128, OSH], bf16)
            nc.sync.dma_start_transpose(ct_sb[:], c_ext[:])
            ucT_sb = small.tile([RK, OSH], bf16)

            b_ap = b_ext[:]
            b_bc = bass.AP(
                tensor=b_ap.tensor,
                offset=b_ap.offset,
                ap=[[0, 128]] + [list(p) for p in b_ap.ap],
            )
            nc.gpsimd.dma_start(out=bias_sb[:], in_=b_bc)

            # PE warmup: keep the array streaming while first DMAs land
            # (p-state ramps to full clock after ~3us of continuous work)
            warm_ps = psA.tile([128, 512], f32, tag="w", bufs=1)
            for _ in range(16):
                nc.tensor.matmul(warm_ps[:], ident[:], cst[:], start=True, stop=True)

            # U^T C^T = (C U)^T : [64 rk, 1024 o]
            for j in range(NJ):
                ps_uc = psA.tile([128, 512], f32, tag="ad", bufs=3, name="ps_uc")
                nc.tensor.matmul(
                    ps_uc[:RK, :],
                    u_sb[:],
                    ct_sb[:RK, ts(j, 512)],
                    start=True,
                    stop=True,
                )
                nc.vector.tensor_copy(out=ucT_sb[:, ts(j, 512)], in_=ps_uc[:RK, :])

            # x^T chunk transposes: ONE xbar instr per chunk,
            # [TC, 4096] -> [128 d, 32 k, TC]; single HWDGE engine (sync) --
            # concurrent transposes on both HWDGE engines corrupt the xbar.
            xt = [
                xtpool.tile([128, NK, TC], bf16, name=f"xt{b}") for b in range(2)
            ]

            def emit_xt_chunk(c):
                nc.sync.dma_start_transpose(xt[c % 2][:], x_ext[ts(c, TC), :])

            # W'^T build: W^T in 4 big xbar transposes of 8 k-slices each,
            # interleaved with early x chunks; adapter (K=64 matmul) added
            # into resident wt_sb by DVE.
            KG = 8  # k-tiles per W transpose group
            for g in range(NK // KG):
                wst = wsp.tile([128, KG, OSH], bf16, name="wst")
                nc.sync.dma_start_transpose(wst[:], w_ext[:, ts(g, KG * 128)])
                if g == 0:
                    emit_xt_chunk(0)
                elif g == 2:
                    emit_xt_chunk(1)
                for kk in range(KG):
                    k = g * KG + kk
                    for j in range(NJ):
                        ps_ad = psA.tile(
                            [128, 512], f32, tag="ad", bufs=3, name="ps_ad"
                        )
                        nc.tensor.matmul(
                            ps_ad[:],
                            r_sb[:, ts(k, 128)],
                            ucT_sb[:, ts(j, 512)],
                            start=True,
                            stop=True,
                        )
                        nc.vector.tensor_add(
                            out=wt_sb[:, k, ts(j, 512)],
                            in0=ps_ad[:],
                            in1=wst[:, kk, ts(j, 512)],
                        )

            # ---------------- main loop ----------------
            for c in range(NCH):
                for tt in range(NTT_C):
                    i = c * NTT_C + tt
                    out_sb = opool.tile([128, OSH], f32, name="out_sb")
                    for j in range(NJ):
                        psm = psB.tile([128, 512], f32, tag="o", bufs=4, name="psm")
                        for k in range(NK):
                            nc.tensor.matmul(
                                psm[:],
                                xt[c % 2][:, k, ts(tt, 128)],
                                wt_sb[:, k, ts(j, 512)],
                                start=(k == 0),
                                stop=(k == NK - 1),
                            )
                        nc.vector.tensor_add(
                            out=out_sb[:, ts(j, 512)],
                            in0=psm[:],
                            in1=bias_sb[:, ts(j, 512)],
                        )
                    nc.gpsimd.dma_start(out=out_ext[ts(i, 128), :], in_=out_sb[:])
                if c + 2 < NCH:
                    emit_xt_chunk(c + 2)
                if _DBG and c == 0:
                    nc.gpsimd.dma_start(
                        out=dbg_xt[:],
                        in_=xt[0][:].rearrange("p k t -> p (k t)"),
                    )
            if _DBG:
                nc.gpsimd.dma_start(
                    out=dbg_wt[:], in_=wt_sb[:].rearrange("p k o -> p (k o)")
                )

    nc.compile()
    return nc


def make_in_maps(x, W, C, U, R, bias):
    bf = ml_dtypes.bfloat16
    x = np.asarray(x, dtype=np.float32).reshape(T, D).astype(bf)
    W = np.asarray(W, dtype=np.float32).astype(bf)
    C = np.asarray(C, dtype=np.float32).astype(bf)
    C_pad = np.zeros((O, 128), dtype=bf)
    C_pad[:, :RK] = C
    U = np.ascontiguousarray(np.asarray(U, dtype=np.float32).astype(bf))
    R = np.ascontiguousarray(np.asarray(R, dtype=np.float32).astype(bf))
    bias = np.ascontiguousarray(np.asarray(bias, dtype=np.float32))

    in_maps = []
    for core in range(N_CORES):
        tg, og = divmod(core, NO)
        in_maps.append(
            {
                "x": np.ascontiguousarray(x[tg * TSH : (tg + 1) * TSH]),
                "W": np.ascontiguousarray(W[og * OSH : (og + 1) * OSH]),
                "C": np.ascontiguousarray(C_pad[og * OSH : (og + 1) * OSH]),
                "U": U,
                "R": R,
                "bias": bias[og * OSH : (og + 1) * OSH],
            }
        )
    return in_maps


def kernel(x, W, C, U, R, bias):
    from concourse.bass_utils import run_bass_kernel_spmd

    in_maps = make_in_maps(x, W, C, U, R, bias)

    if "nc" not in _CACHE:
        _CACHE["nc"] = _build()
    nc = _CACHE["nc"]

    res = run_bass_kernel_spmd(nc, in_maps, core_ids=list(range(N_CORES)))

    out = np.empty((T, O), dtype=np.float32)
    for core in range(N_CORES):
        tg, og = divmod(core, NO)
        out[tg * TSH : (tg + 1) * TSH, og * OSH : (og + 1) * OSH] = res.results[core][
            "out"
        ]
    return out.reshape(B, S, O)
